# revision 1
# baseline (speedup 1.0000x reference)
"""Trainium2 Bass kernel for a 2-layer GAT + mean-pool + linear heads.

Three SPMD launches on 8 NeuronCores; the host performs only integer
indexing / data movement between them (sharding + halo exchange), all
floating-point math runs on device:

  Launch A: hx1[slot] = [x@W1 | a_src1 | a_dst1] for the core's own 5120
            slots (pure matmuls).  Host concatenates the 8 slabs.
  Launch B: layer-1 edge aggregation.  Host feeds, per core, the edge
            streams hx1[src_e] (chunk-major) and a_dst1[dst_e]; device
            does softmax(leaky-relu) attention via one-hot (is_equal)
            matmuls accumulated in PSUM, then h2-table rows
            hx2 = [relu(h1)@W2 | a_src2 | a_dst2].
  Launch C: layer-2 edge aggregation (same pipeline from hx2 streams),
            per-graph mean pooling via one-hot matmuls, AllReduce of the
            [64,129] partial sums across the 8 cores, linear heads.

Nodes are permuted into 320 balanced tiles of 128 slots (greedy by
in-degree) so every tile has <= K*128 incident edges; per-tile edge
lists are padded to exactly K chunks of 128 (pad edges carry
dst_local=-1 and are zeroed by the one-hot).  Softmax omits the
max-subtraction (exact same result; exp arguments are O(10) here).
"""

import os
import sys

sys.path.insert(0, "/opt/trn_rl_repo")

import numpy as np

N = 40000
NP = 40960
C = 8
TPC = 40
NT = C * TPC
SLAB = NP // C            # 5120 slots per core
HEADS, HID = 4, 32
HC = HEADS * HID          # 128
TW = HC + 2 * HEADS       # 136 table row: h | a_src | a_dst
SEG = HC + HEADS          # 132
NEG = 0.2
G = 64                    # graphs

_cache = {}


def _preprocess(edge_index, batch):
    import heapq

    src0 = np.asarray(edge_index[0], dtype=np.int64)
    dst0 = np.asarray(edge_index[1], dtype=np.int64)
    deg = np.bincount(dst0, minlength=N).astype(np.int64) + 1

    order = np.argsort(-deg, kind="stable")
    heap = [(0, 0, t) for t in range(NT)]
    heapq.heapify(heap)
    tile_nodes = [[] for _ in range(NT)]
    for n in order:
        w, ns, t = heapq.heappop(heap)
        tile_nodes[t].append(n)
        if ns + 1 < 128:
            heapq.heappush(heap, (w + deg[n], ns + 1, t))

    slot_of = np.full(N, -1, np.int64)
    node_at = np.full(NP, -1, np.int64)
    for t in range(NT):
        base = t * 128
        nodes = tile_nodes[t]
        slot_of[nodes] = base + np.arange(len(nodes))
        node_at[base:base + len(nodes)] = nodes

    pad_slots = np.where(node_at < 0)[0]
    es = np.concatenate([slot_of[src0], slot_of[np.arange(N)], pad_slots])
    ed = np.concatenate([slot_of[dst0], slot_of[np.arange(N)], pad_slots])
    E = es.shape[0]

    ed_tile = ed >> 7
    order_e = np.argsort(ed_tile, kind="stable")
    es_s, ed_s = es[order_e], ed[order_e]
    counts = np.bincount(ed_tile, minlength=NT)
    offs = np.concatenate([[0], np.cumsum(counts)])
    K = int(np.ceil(counts.max() / 128))

    # per-tile edge lists padded to K*128; pads: src=dst=slot 0, dloc=-1
    est = np.zeros((NT, K * 128), np.int32)
    edt = np.zeros((NT, K * 128), np.int32)
    dloc = np.full((NT, K * 128), -1.0, np.float32)
    pos = np.arange(E) - offs[ed_tile[order_e]]
    est[ed_tile[order_e], pos] = es_s.astype(np.int32)
    edt[ed_tile[order_e], pos] = ed_s.astype(np.int32)
    dloc[ed_tile[order_e], pos] = (ed_s & 127).astype(np.float32)

    # [C, TPC, K, 128] chunk layout (lane = edge % 128)
    est = est.reshape(C, TPC, K, 128)
    edt = edt.reshape(C, TPC, K, 128)
    dloc = dloc.reshape(C, TPC, K, 128)
    # dloc per-core [128, TPC*K] (lane-major) for the device
    import ml_dtypes
    dloc_pc = np.ascontiguousarray(
        dloc.transpose(0, 3, 1, 2).reshape(C, 128, TPC * K)
        .astype(ml_dtypes.bfloat16))

    batch_slot = np.full(NP, -1.0, np.float32)
    real = node_at >= 0
    batch_slot[real] = np.asarray(batch)[node_at[real]].astype(np.float32)
    pool_batch = np.ascontiguousarray(
        batch_slot.reshape(C, TPC, 128).transpose(0, 2, 1))

    return K, node_at, est, edt, dloc_pc, pool_batch


def _block_att(att):
    A = np.zeros((HC, HEADS), np.float32)
    att = np.asarray(att, np.float32)
    for h in range(HEADS):
        A[h * HID:(h + 1) * HID, h] = att[h]
    return A


def _streams_for_core(hx, est_c, edt_c):
    """hx [NP, TW]; est/edt [TPC, K, 128] -> (src bf16 [128, TPC*K*TW],
    ad fp32 [128, TPC*K*HEADS]) lane-major streams."""
    import ml_dtypes
    K = est_c.shape[1]
    g = hx[est_c].astype(ml_dtypes.bfloat16)   # [TPC, K, 128, TW]
    src = np.ascontiguousarray(
        g.transpose(2, 0, 1, 3).reshape(128, TPC * K * TW))
    a = hx[edt_c, SEG:TW]              # [TPC, K, 128, 4]
    ad = np.ascontiguousarray(
        a.transpose(2, 0, 1, 3).reshape(128, TPC * K * HEADS))
    return src, ad


def _bass_mods():
    import concourse.bacc as bacc
    import concourse.mybir as mybir
    import concourse.tile as tile
    import concourse.bass as bass
    return bacc, mybir, tile, bass


def _build_wfull(nc, cp, psA, sbS, ident_t, Wd, Asd, Add, mybir):
    fp32 = mybir.dt.float32
    Ws = sbS.tile([128, HC], fp32, tag="Ws")
    nc.sync.dma_start(out=Ws[:], in_=Wd[:])
    Ast = sbS.tile([128, HEADS], fp32, tag="Ast")
    Adt = sbS.tile([128, HEADS], fp32, tag="Adt")
    nc.sync.dma_start(out=Ast[:], in_=Asd[:])
    nc.sync.dma_start(out=Adt[:], in_=Add[:])
    psT = psA.tile([128, 128], fp32, tag="psT")
    nc.tensor.transpose(out=psT[:], in_=Ws[:], identity=ident_t[:])
    WsT = sbS.tile([128, HC], fp32, tag="WsT")
    nc.vector.tensor_copy(out=WsT[:], in_=psT[:])
    wfull = cp.tile([128, TW], fp32)
    nc.vector.tensor_copy(out=wfull[:, 0:HC], in_=Ws[:])
    psW = psA.tile([128, 2 * HEADS], fp32, tag="psT")
    nc.tensor.matmul(out=psW[:, 0:HEADS], lhsT=WsT[:], rhs=Ast[:],
                     start=True, stop=True)
    nc.tensor.matmul(out=psW[:, HEADS:2 * HEADS], lhsT=WsT[:],
                     rhs=Adt[:], start=True, stop=True)
    nc.vector.tensor_copy(out=wfull[:, HC:TW], in_=psW[:])
    return wfull


def _build_A():
    """Launch A: hx1 rows for the core's 5120 slots."""
    bacc, mybir, tile, bass = _bass_mods()
    fp32 = mybir.dt.float32
    nc = bacc.Bacc("TRN2", target_bir_lowering=False, debug=False,
                   num_devices=C)
    x_loc = nc.dram_tensor("x_loc", [SLAB, HC], fp32, kind="ExternalInput")
    W1d = nc.dram_tensor("W1", [HC, HC], fp32, kind="ExternalInput")
    As1 = nc.dram_tensor("As1", [HC, HEADS], fp32, kind="ExternalInput")
    Ad1 = nc.dram_tensor("Ad1", [HC, HEADS], fp32, kind="ExternalInput")
    identD = nc.dram_tensor("ident128", [128, 128], fp32, kind="ExternalInput")
    outD = nc.dram_tensor("hx1_loc", [SLAB, TW], fp32, kind="ExternalOutput")

    with tile.TileContext(nc) as tc:
        with tc.tile_pool(name="const", bufs=1) as cp, \
             tc.tile_pool(name="sbA", bufs=4) as sbA, \
             tc.tile_pool(name="sbS", bufs=2) as sbS, \
             tc.tile_pool(name="psA", bufs=2, space="PSUM") as psA:
            ident_t = cp.tile([128, 128], fp32)
            nc.sync.dma_start(out=ident_t[:], in_=identD[:])
            wfull1 = _build_wfull(nc, cp, psA, sbS, ident_t,
                                  W1d, As1, Ad1, mybir)
            for t in range(TPC):
                xt = sbA.tile([128, HC], fp32, tag="xt")
                nc.sync.dma_start(out=xt[:],
                                  in_=x_loc[t * 128:(t + 1) * 128, :])
                psT = psA.tile([128, 128], fp32, tag="psT2")
                nc.tensor.transpose(out=psT[:], in_=xt[:],
                                    identity=ident_t[:])
                xT = sbA.tile([128, 128], fp32, tag="xT")
                nc.vector.tensor_copy(out=xT[:], in_=psT[:])
                psH = psA.tile([128, TW], fp32, tag="psH")
                nc.tensor.matmul(out=psH[:], lhsT=xT[:], rhs=wfull1[:],
                                 start=True, stop=True)
                hxt = sbA.tile([128, TW], fp32, tag="hxt")
                nc.vector.tensor_copy(out=hxt[:], in_=psH[:])
                nc.sync.dma_start(out=outD[t * 128:(t + 1) * 128, :],
                                  in_=hxt[:])
    nc.compile()
    return nc


def _edge_layer(nc, pools, K, srcD, adD, dlocD, bias_t, mybir,
                per_tile_post, hr_ones_col=False, GT=2):
    """Per-tile: stream per-edge hx rows, one-hot (is_equal) build, attention
    P, PSUM-accumulated aggregation, normalize + bias + relu, then
    per_tile_post(t, h_r)."""
    fp32 = mybir.dt.float32
    bf16 = mybir.dt.bfloat16
    OP = mybir.AluOpType
    AF = mybir.ActivationFunctionType
    cp, sbB, sbS, psU, iota_t = pools
    CW = K * TW

    dloc_t = cp.tile([128, TPC * K], bf16)
    nc.sync.dma_start(out=dloc_t[:], in_=dlocD[:])
    iota_b = cp.tile([128, 128], bf16)
    nc.vector.tensor_copy(out=iota_b[:], in_=iota_t[:])

    for g in range(TPC // GT):
        Hg = sbB.tile([128, GT * K * TW], bf16, tag="Hg")
        nc.sync.dma_start(out=Hg[:], in_=srcD[:, g * GT * CW:(g + 1) * GT * CW])
        Adg = sbS.tile([128, GT * K * HEADS], fp32, tag="Adg")
        nc.sync.dma_start(
            out=Adg[:],
            in_=adD[:, g * GT * K * HEADS:(g + 1) * GT * K * HEADS])

        a_s_view = Hg[:].rearrange("p (k s) -> p k s", s=TW)[:, :, HC:SEG]
        P = sbS.tile([128, GT * K * HEADS], fp32, tag="P")
        nc.vector.tensor_tensor(out=P[:], in0=a_s_view, in1=Adg[:], op=OP.add)
        nc.vector.scalar_tensor_tensor(out=P[:], in0=P[:], scalar=NEG,
                                       in1=P[:], op0=OP.mult, op1=OP.max)
        nc.scalar.activation(P[:], P[:], AF.Exp)

        eqT = sbB.tile([128, GT * K * 128], bf16, tag="eqT")
        cs, ce = g * GT * K, (g + 1) * GT * K
        dl_b = dloc_t[:, cs:ce].to_broadcast([128, GT * K, 128])
        io_b = iota_b[:].rearrange("p (o d) -> p o d", o=1) \
            .to_broadcast([128, GT * K, 128])
        nc.vector.tensor_tensor(out=eqT[:], in0=dl_b, in1=io_b,
                                op=OP.is_equal)

        P2 = sbS.tile([128, GT * K * HEADS], bf16, tag="P2")
        nc.vector.tensor_copy(out=P2[:], in_=P[:])
        h_view = Hg[:].rearrange("p (k s) -> p k s", s=TW)[:, :, 0:HC]
        p_rep = P2[:].rearrange("p (k h) -> p k h", h=HEADS) \
            .to_broadcast([128, GT * K, HEADS, HID])
        nc.vector.tensor_tensor(out=h_view, in0=h_view, in1=p_rep,
                                op=OP.mult)
        nc.vector.tensor_copy(out=a_s_view, in_=P2[:])

        for j in range(GT):
            t = g * GT + j
            U = psU.tile([128, SEG], fp32, tag="U", space="PSUM")
            for k in range(K):
                kk = j * K + k
                nc.tensor.matmul(out=U[:],
                                 lhsT=eqT[:, kk * 128:(kk + 1) * 128],
                                 rhs=Hg[:, kk * TW:kk * TW + SEG],
                                 start=(k == 0), stop=(k == K - 1))

            rec = sbS.tile([128, HEADS], fp32, tag="rec")
            nc.vector.reciprocal(rec[:], U[:, HC:SEG])
            hw = HC + 1 if hr_ones_col else HC
            hr = sbB.tile([128, hw], fp32, tag="hr")
            rec_rep = rec[:].to_broadcast([128, HEADS, HID])
            nc.vector.tensor_tensor(
                out=hr[:, 0:HC].rearrange("p (h c) -> p h c", h=HEADS),
                in0=U[:, 0:HC].rearrange("p (h c) -> p h c", h=HEADS),
                in1=rec_rep, op=OP.mult)
            nc.vector.tensor_tensor(out=hr[:, 0:HC], in0=hr[:, 0:HC],
                                    in1=bias_t[:], op=OP.add)
            nc.scalar.activation(hr[:, 0:HC], hr[:, 0:HC], AF.Relu)
            if hr_ones_col:
                nc.vector.memset(hr[:, HC:HC + 1], 1.0)
            per_tile_post(t, hr)


def _build_B(K):
    """Launch B: layer-1 edges -> hx2 rows."""
    bacc, mybir, tile, bass = _bass_mods()
    fp32 = mybir.dt.float32
    nc = bacc.Bacc("TRN2", target_bir_lowering=False, debug=False,
                   num_devices=C)
    srcD = nc.dram_tensor("src_stream", [128, TPC * K * TW],
                          mybir.dt.bfloat16, kind="ExternalInput")
    adD = nc.dram_tensor("ad_stream", [128, TPC * K * HEADS], fp32,
                         kind="ExternalInput")
    dlocD = nc.dram_tensor("dloc", [128, TPC * K], mybir.dt.bfloat16,
                           kind="ExternalInput")
    b1B = nc.dram_tensor("b1B", [128, HC], fp32, kind="ExternalInput")
    W2d = nc.dram_tensor("W2", [HC, HC], fp32, kind="ExternalInput")
    As2 = nc.dram_tensor("As2", [HC, HEADS], fp32, kind="ExternalInput")
    Ad2 = nc.dram_tensor("Ad2", [HC, HEADS], fp32, kind="ExternalInput")
    iotaD = nc.dram_tensor("iota128", [128, 128], fp32, kind="ExternalInput")
    identD = nc.dram_tensor("ident128", [128, 128], fp32,
                            kind="ExternalInput")
    outD = nc.dram_tensor("hx2_loc", [SLAB, TW], fp32, kind="ExternalOutput")

    with tile.TileContext(nc) as tc:
        with tc.tile_pool(name="const", bufs=1) as cp, \
             tc.tile_pool(name="sbB", bufs=3) as sbB, \
             tc.tile_pool(name="sbS", bufs=3) as sbS, \
             tc.tile_pool(name="sbA", bufs=3) as sbA, \
             tc.tile_pool(name="psA", bufs=2, space="PSUM") as psA, \
             tc.tile_pool(name="psU", bufs=2, space="PSUM") as psU:
            iota_t = cp.tile([128, 128], fp32)
            ident_t = cp.tile([128, 128], fp32)
            bias1 = cp.tile([128, HC], fp32)
            nc.sync.dma_start(out=iota_t[:], in_=iotaD[:])
            nc.sync.dma_start(out=ident_t[:], in_=identD[:])
            nc.sync.dma_start(out=bias1[:], in_=b1B[:])
            wfull2 = _build_wfull(nc, cp, psA, sbS, ident_t,
                                  W2d, As2, Ad2, mybir)

            def post(t, h1r):
                psT = psA.tile([128, 128], fp32, tag="psT2")
                nc.tensor.transpose(out=psT[:], in_=h1r[:],
                                    identity=ident_t[:])
                hT = sbA.tile([128, 128], fp32, tag="hT")
                nc.vector.tensor_copy(out=hT[:], in_=psT[:])
                psH = psA.tile([128, TW], fp32, tag="psH")
                nc.tensor.matmul(out=psH[:], lhsT=hT[:], rhs=wfull2[:],
                                 start=True, stop=True)
                hxt = sbA.tile([128, TW], fp32, tag="hxt")
                nc.vector.tensor_copy(out=hxt[:], in_=psH[:])
                nc.sync.dma_start(out=outD[t * 128:(t + 1) * 128, :],
                                  in_=hxt[:])

            _edge_layer(nc, (cp, sbB, sbS, psU, iota_t), K,
                        srcD, adD, dlocD, bias1, mybir, post)
    nc.compile()
    return nc


def _build_C(K):
    """Launch C: layer-2 edges -> pooling -> AllReduce -> heads."""
    bacc, mybir, tile, bass = _bass_mods()
    fp32 = mybir.dt.float32
    OP = mybir.AluOpType
    nc = bacc.Bacc("TRN2", target_bir_lowering=False, debug=False,
                   num_devices=C)
    srcD = nc.dram_tensor("src_stream", [128, TPC * K * TW],
                          mybir.dt.bfloat16, kind="ExternalInput")
    adD = nc.dram_tensor("ad_stream", [128, TPC * K * HEADS], fp32,
                         kind="ExternalInput")
    dlocD = nc.dram_tensor("dloc", [128, TPC * K], mybir.dt.bfloat16,
                           kind="ExternalInput")
    b2B = nc.dram_tensor("b2B", [128, HC], fp32, kind="ExternalInput")
    pbD = nc.dram_tensor("pool_batch", [128, TPC], fp32,
                         kind="ExternalInput")
    WrB = nc.dram_tensor("WrB", [G, HC], fp32, kind="ExternalInput")
    WtB = nc.dram_tensor("WtB", [G, HC], fp32, kind="ExternalInput")
    brB = nc.dram_tensor("brB", [G, 1], fp32, kind="ExternalInput")
    btB = nc.dram_tensor("btB", [G, 1], fp32, kind="ExternalInput")
    iotaD = nc.dram_tensor("iota128", [128, 128], fp32, kind="ExternalInput")
    outD = nc.dram_tensor("out", [G, 2], fp32, kind="ExternalOutput")

    with tile.TileContext(nc) as tc:
        with tc.tile_pool(name="const", bufs=1) as cp, \
             tc.tile_pool(name="sbB", bufs=4) as sbB, \
             tc.tile_pool(name="sbS", bufs=4) as sbS, \
             tc.tile_pool(name="psU", bufs=4, space="PSUM") as psU, \
             tc.tile_pool(name="psP", bufs=1, space="PSUM") as psP, \
             tc.tile_pool(name="dram", bufs=1, space="DRAM") as dram:
            iota_t = cp.tile([128, 128], fp32)
            bias2 = cp.tile([128, HC], fp32)
            pb_t = cp.tile([128, TPC], fp32)
            nc.sync.dma_start(out=iota_t[:], in_=iotaD[:])
            nc.sync.dma_start(out=bias2[:], in_=b2B[:])
            nc.sync.dma_start(out=pb_t[:], in_=pbD[:])
            ones_col = cp.tile([128, 1], fp32)
            nc.vector.memset(ones_col[:], 1.0)

            pool_ps = psP.tile([G, HC + 1], fp32, tag="poolps", space="PSUM")

            def post(t, h2r):
                eqg = sbS.tile([128, G], fp32, tag="eqg")
                pb_b = pb_t[:, t:t + 1].to_broadcast([128, 1, G])
                io_b = iota_t[:, 0:G].rearrange("p (o d) -> p o d", o=1)
                nc.vector.tensor_tensor(
                    out=eqg[:].rearrange("p (o g) -> p o g", o=1),
                    in0=pb_b, in1=io_b, op=OP.is_equal)
                nc.tensor.matmul(out=pool_ps[:], lhsT=eqg[:],
                                 rhs=h2r[:],
                                 start=(t == 0), stop=(t == TPC - 1))

            _edge_layer(nc, (cp, sbB, sbS, psU, iota_t), K,
                        srcD, adD, dlocD, bias2, mybir, post,
                        hr_ones_col=True, GT=1)

            pool_sb = sbS.tile([G, HC + 1], fp32, tag="poolsb")
            nc.vector.tensor_copy(out=pool_sb[:], in_=pool_ps[:])
            ar_in = dram.tile([G, HC + 1], fp32)
            ar_out = dram.tile([G, HC + 1], fp32)
            nc.sync.dma_start(out=ar_in[:], in_=pool_sb[:])
            nc.gpsimd.collective_compute(
                "AllReduce", mybir.AluOpType.add,
                replica_groups=[list(range(C))],
                ins=[ar_in.opt()], outs=[ar_out.opt()])
            AR = sbS.tile([G, HC + 1], fp32, tag="AR")
            nc.sync.dma_start(out=AR[:], in_=ar_out[:])

            WrT = cp.tile([G, HC], fp32)
            WtT = cp.tile([G, HC], fp32)
            brT = cp.tile([G, 1], fp32)
            btT = cp.tile([G, 1], fp32)
            nc.sync.dma_start(out=WrT[:], in_=WrB[:])
            nc.sync.dma_start(out=WtT[:], in_=WtB[:])
            nc.sync.dma_start(out=brT[:], in_=brB[:])
            nc.sync.dma_start(out=btT[:], in_=btB[:])

            recC = sbS.tile([G, 1], fp32, tag="recC")
            nc.vector.reciprocal(recC[:], AR[:, HC:HC + 1])
            pooled = sbS.tile([G, HC], fp32, tag="pooled")
            nc.vector.tensor_tensor(out=pooled[:], in0=AR[:, 0:HC],
                                    in1=recC[:].to_broadcast([G, HC]),
                                    op=OP.mult)
            out_t = sbS.tile([G, 2], fp32, tag="outt")
            for j, Wt_ in enumerate([WrT, WtT]):
                prod = sbS.tile([G, HC], fp32, tag="prod")
                nc.vector.tensor_tensor(out=prod[:], in0=pooled[:],
                                        in1=Wt_[:], op=OP.mult)
                nc.vector.tensor_reduce(out=out_t[:, j:j + 1], in_=prod[:],
                                        axis=mybir.AxisListType.X, op=OP.add)
            nc.vector.tensor_tensor(out=out_t[:, 0:1], in0=out_t[:, 0:1],
                                    in1=brT[:], op=OP.add)
            nc.vector.tensor_tensor(out=out_t[:, 1:2], in0=out_t[:, 1:2],
                                    in1=btT[:], op=OP.add)
            nc.sync.dma_start(out=outD[:], in_=out_t[:])
    nc.compile()
    return nc


def _run(nc, in_maps, trace):
    from concourse.bass_utils import run_bass_kernel_spmd
    return run_bass_kernel_spmd(nc, in_maps, core_ids=list(range(C)),
                                trace=trace)


def kernel(**inputs):
    x = np.asarray(inputs["x"], np.float32)
    edge_index = np.asarray(inputs["edge_index"])
    batch = np.asarray(inputs["batch"])

    K, node_at, est, edt, dloc_pc, pool_batch = _preprocess(edge_index, batch)
    if _cache.get("K") != K:
        _cache.clear()
        _cache["K"] = K
        _cache["A"] = _build_A()
        _cache["B"] = _build_B(K)
        _cache["C"] = _build_C(K)
    ncA, ncB, ncC = _cache["A"], _cache["B"], _cache["C"]

    x_perm = np.zeros((NP, HC), np.float32)
    real = node_at >= 0
    x_perm[real] = x[node_at[real]]

    iota128 = np.ascontiguousarray(
        np.broadcast_to(np.arange(128, dtype=np.float32), (128, 128)))
    ident128 = np.eye(128, dtype=np.float32)
    b1B = np.ascontiguousarray(np.broadcast_to(
        np.asarray(inputs["b1"], np.float32), (128, HC)))
    b2B = np.ascontiguousarray(np.broadcast_to(
        np.asarray(inputs["b2"], np.float32), (128, HC)))
    WrB = np.ascontiguousarray(np.broadcast_to(
        np.asarray(inputs["Wr"], np.float32).reshape(1, HC), (G, HC)))
    WtB = np.ascontiguousarray(np.broadcast_to(
        np.asarray(inputs["Wt"], np.float32).reshape(1, HC), (G, HC)))
    brB = np.ascontiguousarray(np.broadcast_to(
        np.asarray(inputs["br"], np.float32).reshape(1, 1), (G, 1)))
    btB = np.ascontiguousarray(np.broadcast_to(
        np.asarray(inputs["bt"], np.float32).reshape(1, 1), (G, 1)))

    trace = os.environ.get("GAT_TRACE", "0") == "1"
    if trace:
        _install_ntff_shim()
    times = []

    # ---- launch A ----
    mapsA = []
    for c in range(C):
        mapsA.append({
            "x_loc": np.ascontiguousarray(x_perm[c * SLAB:(c + 1) * SLAB]),
            "W1": np.asarray(inputs["W1"], np.float32),
            "As1": _block_att(inputs["att_src1"]),
            "Ad1": _block_att(inputs["att_dst1"]),
            "ident128": ident128,
        })
    resA = _run(ncA, mapsA, trace)
    times.append(resA.exec_time_ns)
    hx1 = np.concatenate([resA.results[c]["hx1_loc"] for c in range(C)])

    # ---- launch B ----
    mapsB = []
    for c in range(C):
        src, ad = _streams_for_core(hx1, est[c], edt[c])
        mapsB.append({
            "src_stream": src, "ad_stream": ad, "dloc": dloc_pc[c],
            "b1B": b1B,
            "W2": np.asarray(inputs["W2"], np.float32),
            "As2": _block_att(inputs["att_src2"]),
            "Ad2": _block_att(inputs["att_dst2"]),
            "iota128": iota128, "ident128": ident128,
        })
    resB = _run(ncB, mapsB, trace)
    times.append(resB.exec_time_ns)
    hx2 = np.concatenate([resB.results[c]["hx2_loc"] for c in range(C)])

    # ---- launch C ----
    mapsC = []
    for c in range(C):
        src, ad = _streams_for_core(hx2, est[c], edt[c])
        mapsC.append({
            "src_stream": src, "ad_stream": ad, "dloc": dloc_pc[c],
            "b2B": b2B, "pool_batch": pool_batch[c],
            "WrB": WrB, "WtB": WtB, "brB": brB, "btB": btB,
            "iota128": iota128,
        })
    resC = _run(ncC, mapsC, trace)
    times.append(resC.exec_time_ns)

    kernel._last_exec_times_ns = times
    kernel._last_exec_time_ns = (sum(t for t in times if t is not None)
                                 if any(t is not None for t in times) else None)
    return np.asarray(resC.results[0]["out"])


kernel._last_exec_time_ns = None
kernel._last_exec_times_ns = None


def _install_ntff_shim():
    import types
    if "antenv.axon_hooks" in sys.modules:
        return
    try:
        from trn_agent_boot.trn_boot import _ntff_profile_via_ctypes
        hook = _ntff_profile_via_ctypes("/opt/axon/libaxon_pjrt.so")
    except Exception:
        hook = None
    mod = types.ModuleType("antenv.axon_hooks")
    mod.get_axon_ntff_profile_hook = lambda: hook
    mod.set_axon_ntff_profile_hook = lambda h: None
    sys.modules["antenv.axon_hooks"] = mod



# revision 2
# speedup vs baseline: 1.4350x; 1.4350x over previous
"""Trainium2 Bass kernel for a 2-layer GAT + mean-pool + linear heads.

Three SPMD launches on 8 NeuronCores; the host performs only integer
indexing / data movement between them (sharding + gathers), all
floating-point math runs on device:

  Launch A: hx1[slot] = [x@W1 | a_src1 | a_dst1] for the core's own 5120
            slots (host supplies x pre-transposed, so A is pure matmuls).
  Launch B: layer-1 edge aggregation.  Host feeds per-edge streams
            h1[src_e] (chunk-major, 132-wide slots with a 4-col gap for
            the softmax weights), a_src1[src_e], a_dst1[dst_e], and a
            HOST-BUILT fp8 one-hot of the local dst slot.  Device does
            softmax(leaky-relu) attention: exp runs on the scalar engine
            twice -- once 4-wide into the chunk gap (denominator cols),
            once broadcast-expanded to 128-wide (numerator weights) --
            then a packed bf16 multiply and one-hot matmuls accumulated
            in PSUM.  Normalize+bias+relu, then hx2 rows via W2.
  Launch C: layer-2 edge aggregation (same pipeline from hx2 streams),
            per-graph mean pooling via a host-built one-hot, AllReduce
            of the [64,128] partial sums, divide by host-known counts,
            linear heads.

Nodes are permuted into 640 balanced tiles of 64 slots (greedy by
in-degree); per-tile edge lists are padded to exactly K chunks of 128
(pad edges carry an all-zero one-hot column).  Softmax omits the
max-subtraction (exact same result; exp arguments are O(10) here).
"""

import os
import sys

sys.path.insert(0, "/opt/trn_rl_repo")

import numpy as np

N = 40000
NP = 40960
C = 8
SLAB = NP // C            # 5120 slots per core
HEADS, HID = 4, 32
HC = HEADS * HID          # 128
TW = HC + 2 * HEADS       # 136 table row: h | a_src | a_dst
CW = HC + HEADS           # 132 chunk width: h | p4
NEG = 0.2
G = 64                    # graphs

TQ = 64                   # dst slots per tile
NT = NP // TQ             # 640 tiles
TPC = NT // C             # 80 tiles per core
GRP = 128 // TQ           # tiles per norm-group (2)
NG = TPC // GRP           # 40 groups per core
EXP_ACT = 14              # chunks per group expanded on scalar engine
                          # (rest on vector engine); tuned by trace

_cache = {}


def _preprocess(edge_index, batch):
    import heapq

    src0 = np.asarray(edge_index[0], dtype=np.int64)
    dst0 = np.asarray(edge_index[1], dtype=np.int64)
    deg = np.bincount(dst0, minlength=N).astype(np.int64) + 1

    order = np.argsort(-deg, kind="stable")
    heap = [(0, 0, t) for t in range(NT)]
    heapq.heapify(heap)
    tile_nodes = [[] for _ in range(NT)]
    for n in order:
        w, ns, t = heapq.heappop(heap)
        tile_nodes[t].append(n)
        if ns + 1 < TQ:
            heapq.heappush(heap, (w + deg[n], ns + 1, t))

    slot_of = np.full(N, -1, np.int64)
    node_at = np.full(NP, -1, np.int64)
    for t in range(NT):
        base = t * TQ
        nodes = tile_nodes[t]
        slot_of[nodes] = base + np.arange(len(nodes))
        node_at[base:base + len(nodes)] = nodes

    pad_slots = np.where(node_at < 0)[0]
    es = np.concatenate([slot_of[src0], slot_of[np.arange(N)], pad_slots])
    ed = np.concatenate([slot_of[dst0], slot_of[np.arange(N)], pad_slots])
    E = es.shape[0]

    ed_tile = ed // TQ
    order_e = np.argsort(ed_tile, kind="stable")
    es_s, ed_s = es[order_e], ed[order_e]
    counts = np.bincount(ed_tile, minlength=NT)
    offs = np.concatenate([[0], np.cumsum(counts)])
    K = int(np.ceil(counts.max() / 128))

    # per-tile edge lists padded to K*128; pads: src=dst=slot 0, dloc=-1
    est = np.zeros((NT, K * 128), np.int32)
    edt = np.zeros((NT, K * 128), np.int32)
    dloc = np.full((NT, K * 128), -1, np.int32)
    pos = np.arange(E) - offs[ed_tile[order_e]]
    est[ed_tile[order_e], pos] = es_s.astype(np.int32)
    edt[ed_tile[order_e], pos] = ed_s.astype(np.int32)
    dloc[ed_tile[order_e], pos] = (ed_s % TQ).astype(np.int32)

    est = est.reshape(C, TPC, K, 128)
    edt = edt.reshape(C, TPC, K, 128)
    dloc = dloc.reshape(C, TPC, K, 128)

    import ml_dtypes
    # host-built one-hot of dst-local slot, fp8 (values 0/1 exact),
    # lane-major: [C, 128, TPC*K*TQ]
    oh = (dloc[..., None] == np.arange(TQ, dtype=np.int32))
    oh_pc = np.ascontiguousarray(
        oh.transpose(0, 3, 1, 2, 4).reshape(C, 128, TPC * K * TQ)
        .astype(ml_dtypes.float8_e4m3))

    # pooling one-hot per 128-slot norm group: [C, 128, NG*G]
    batch_slot = np.full(NP, -1, np.int64)
    real = node_at >= 0
    batch_slot[real] = np.asarray(batch)[node_at[real]]
    bs = batch_slot.reshape(C, NG, 128)
    po = (bs[..., None] == np.arange(G, dtype=np.int64))   # [C, NG, 128, G]
    po_pc = np.ascontiguousarray(
        po.transpose(0, 2, 1, 3).reshape(C, 128, NG * G)
        .astype(ml_dtypes.bfloat16))

    cnt = np.maximum(np.bincount(np.asarray(batch), minlength=G), 1)
    cnt = np.ascontiguousarray(cnt.astype(np.float32).reshape(G, 1))

    return K, node_at, est, edt, oh_pc, po_pc, cnt


def _block_att(att):
    A = np.zeros((HC, HEADS), np.float32)
    att = np.asarray(att, np.float32)
    for h in range(HEADS):
        A[h * HID:(h + 1) * HID, h] = att[h]
    return A


def _streams_for_core(hx, est_c, edt_c):
    """hx [NP, TW] bf16; est/edt [TPC, K, 128] -> (h bf16 [128, NCH*CW],
    as bf16 [128, NCH*4], ad bf16 [128, NCH*4]) lane-major streams."""
    import ml_dtypes
    K = est_c.shape[1]
    g = hx[est_c]                                # [TPC, K, 128, TW] bf16
    z = np.zeros((TPC, K, 128, CW), ml_dtypes.bfloat16)
    z[..., 0:HC] = g[..., 0:HC]
    h_s = np.ascontiguousarray(
        z.transpose(2, 0, 1, 3).reshape(128, TPC * K * CW))
    as_s = np.ascontiguousarray(
        g[..., HC:HC + HEADS].transpose(2, 0, 1, 3)
        .reshape(128, TPC * K * HEADS))
    ad_s = np.ascontiguousarray(
        hx[edt_c][..., HC + HEADS:TW].transpose(2, 0, 1, 3)
        .reshape(128, TPC * K * HEADS))
    return h_s, as_s, ad_s


def _bass_mods():
    import concourse.bacc as bacc
    import concourse.mybir as mybir
    import concourse.tile as tile
    import concourse.bass as bass
    return bacc, mybir, tile, bass


def _build_wfull(nc, cp, psA, sbS, ident_t, Wd, Asd, Add, mybir):
    """wfull bf16 [128, TW] = [W | W@As_blk | W@Ad_blk]."""
    fp32 = mybir.dt.float32
    bf16 = mybir.dt.bfloat16
    Ws = sbS.tile([128, HC], fp32, tag="Ws")
    nc.sync.dma_start(out=Ws[:], in_=Wd[:])
    Ast = sbS.tile([128, HEADS], fp32, tag="Ast")
    Adt = sbS.tile([128, HEADS], fp32, tag="Adt")
    nc.sync.dma_start(out=Ast[:], in_=Asd[:])
    nc.sync.dma_start(out=Adt[:], in_=Add[:])
    psT = psA.tile([128, 128], fp32, tag="psT")
    nc.tensor.transpose(out=psT[:], in_=Ws[:], identity=ident_t[:])
    WsT = sbS.tile([128, HC], fp32, tag="WsT")
    nc.vector.tensor_copy(out=WsT[:], in_=psT[:])
    wfull = cp.tile([128, TW], bf16)
    nc.vector.tensor_copy(out=wfull[:, 0:HC], in_=Ws[:])
    psW = psA.tile([128, 2 * HEADS], fp32, tag="psT")
    nc.tensor.matmul(out=psW[:, 0:HEADS], lhsT=WsT[:], rhs=Ast[:],
                     start=True, stop=True)
    nc.tensor.matmul(out=psW[:, HEADS:2 * HEADS], lhsT=WsT[:],
                     rhs=Adt[:], start=True, stop=True)
    nc.vector.tensor_copy(out=wfull[:, HC:TW], in_=psW[:])
    return wfull


def _build_A():
    """Launch A: hx1 rows for the core's 5120 slots (host-transposed x)."""
    bacc, mybir, tile, bass = _bass_mods()
    fp32 = mybir.dt.float32
    bf16 = mybir.dt.bfloat16
    nc = bacc.Bacc("TRN2", target_bir_lowering=False, debug=False,
                   num_devices=C)
    xTd = nc.dram_tensor("xT", [128, SLAB], bf16, kind="ExternalInput")
    W1d = nc.dram_tensor("W1", [HC, HC], fp32, kind="ExternalInput")
    As1 = nc.dram_tensor("As1", [HC, HEADS], fp32, kind="ExternalInput")
    Ad1 = nc.dram_tensor("Ad1", [HC, HEADS], fp32, kind="ExternalInput")
    identD = nc.dram_tensor("ident128", [128, 128], fp32, kind="ExternalInput")
    outD = nc.dram_tensor("hx1_loc", [SLAB, TW], bf16, kind="ExternalOutput")

    with tile.TileContext(nc) as tc:
        with tc.tile_pool(name="const", bufs=1) as cp, \
             tc.tile_pool(name="sbA", bufs=4) as sbA, \
             tc.tile_pool(name="sbS", bufs=2) as sbS, \
             tc.tile_pool(name="psA", bufs=4, space="PSUM") as psA:
            ident_t = cp.tile([128, 128], fp32)
            nc.sync.dma_start(out=ident_t[:], in_=identD[:])
            wfull1 = _build_wfull(nc, cp, psA, sbS, ident_t,
                                  W1d, As1, Ad1, mybir)
            xT = cp.tile([128, SLAB], bf16)
            nc.sync.dma_start(out=xT[:], in_=xTd[:])
            for t in range(SLAB // 128):
                psH = psA.tile([128, TW], fp32, tag="psH")
                nc.tensor.matmul(out=psH[:],
                                 lhsT=xT[:, t * 128:(t + 1) * 128],
                                 rhs=wfull1[:], start=True, stop=True)
                hxt = sbA.tile([128, TW], bf16, tag="hxt")
                nc.vector.tensor_copy(out=hxt[:], in_=psH[:])
                nc.sync.dma_start(out=outD[t * 128:(t + 1) * 128, :],
                                  in_=hxt[:])
    nc.compile()
    return nc


def _edge_layer(nc, pools, K, hD, asD, adD, ohD, bias_t, mybir,
                per_group_post):
    """Per norm-group (128 dst slots = GRP tiles, GC = GRP*K chunks):
    stream per-edge h rows + attention terms, softmax weights via
    scalar-engine exp (4-wide for denominators + broadcast-expanded
    128-wide for numerators), packed bf16 multiply, one-hot matmuls
    accumulated in PSUM, normalize + bias + relu, then
    per_group_post(g, h1f [128,128] fp32)."""
    fp32 = mybir.dt.float32
    bf16 = mybir.dt.bfloat16
    fp8 = mybir.dt.float8e4
    OP = mybir.AluOpType
    AF = mybir.ActivationFunctionType
    cp, sbH, sbO, sbP, sbS, psU = pools
    GC = GRP * K                      # chunks per group

    As_t = cp.tile([128, TPC * K * HEADS], bf16)
    Ad_t = cp.tile([128, TPC * K * HEADS], bf16)
    nc.sync.dma_start(out=As_t[:], in_=asD[:])
    nc.sync.dma_start(out=Ad_t[:], in_=adD[:])

    for g in range(NG):
        Hseg = sbH.tile([128, GC * CW], bf16, tag="Hseg")
        nc.sync.dma_start(out=Hseg[:],
                          in_=hD[:, g * GC * CW:(g + 1) * GC * CW])
        oh_t = sbO.tile([128, GC * TQ], fp8, tag="oh")
        nc.sync.dma_start(out=oh_t[:],
                          in_=ohD[:, g * GC * TQ:(g + 1) * GC * TQ])

        cs, ce = g * GC * HEADS, (g + 1) * GC * HEADS
        S = sbS.tile([128, GC * HEADS], bf16, tag="S")
        nc.vector.tensor_tensor(out=S[:], in0=As_t[:, cs:ce],
                                in1=Ad_t[:, cs:ce], op=OP.add)
        nc.vector.scalar_tensor_tensor(out=S[:], in0=S[:], scalar=NEG,
                                       in1=S[:], op0=OP.mult, op1=OP.max)

        # denominator columns: exp into the 4-col gaps of each chunk
        hs_v = Hseg[:].rearrange("p (k s) -> p k s", s=CW)
        s_v = S[:].rearrange("p (k h) -> p k h", h=HEADS)
        nc.scalar.activation(hs_v[:, :, HC:CW], s_v, AF.Exp)

        # numerator weights: exp broadcast-expanded to 128 wide
        pexp = sbP.tile([128, GC * HC], bf16, tag="pexp")
        pe_v = pexp[:].rearrange("p (k h c) -> p k h c", h=HEADS, c=HID)
        na = EXP_ACT
        nc.scalar.activation(
            pe_v[:, 0:na],
            s_v[:, 0:na].to_broadcast([128, na, HEADS, HID]), AF.Exp)
        if na < GC:
            # vector-engine share: exp already applied values are not
            # available; recompute exp on DVE is not possible -- instead
            # broadcast-copy from the denominator slots (already exp'ed)
            nc.vector.tensor_copy(
                out=pe_v[:, na:GC],
                in_=hs_v[:, na:GC, HC:CW]
                .to_broadcast([128, GC - na, HEADS, HID]))

        # numerator: h *= p  (packed bf16, unit-stride innermost)
        h_v = hs_v[:, :, 0:HC]
        nc.vector.tensor_tensor(
            out=h_v, in0=h_v,
            in1=pexp[:].rearrange("p (k c) -> p k c", c=HC), op=OP.mult)

        U = psU.tile([128, CW], fp32, tag="U", space="PSUM")
        for j in range(GRP):
            for k in range(K):
                kk = j * K + k
                nc.tensor.matmul(out=U[j * TQ:(j + 1) * TQ, :],
                                 lhsT=oh_t[:, kk * TQ:(kk + 1) * TQ],
                                 rhs=Hseg[:, kk * CW:(kk + 1) * CW],
                                 start=(k == 0), stop=(k == K - 1))

        rec = sbS.tile([128, HEADS], fp32, tag="rec")
        nc.vector.reciprocal(rec[:], U[:, HC:CW])
        h1f = sbS.tile([128, HC], fp32, tag="h1f")
        nc.vector.tensor_tensor(
            out=h1f[:].rearrange("p (h c) -> p h c", h=HEADS),
            in0=U[:, 0:HC].rearrange("p (h c) -> p h c", h=HEADS),
            in1=rec[:].to_broadcast([128, HEADS, HID]), op=OP.mult)
        nc.vector.tensor_tensor(out=h1f[:], in0=h1f[:], in1=bias_t[:],
                                op=OP.add)
        nc.vector.tensor_scalar_max(out=h1f[:], in0=h1f[:], scalar1=0.0)
        per_group_post(g, h1f)


def _build_B(K):
    """Launch B: layer-1 edges -> hx2 rows."""
    bacc, mybir, tile, bass = _bass_mods()
    fp32 = mybir.dt.float32
    bf16 = mybir.dt.bfloat16
    fp8 = mybir.dt.float8e4
    NCH = TPC * K
    nc = bacc.Bacc("TRN2", target_bir_lowering=False, debug=False,
                   num_devices=C)
    hD = nc.dram_tensor("h_stream", [128, NCH * CW], bf16,
                        kind="ExternalInput")
    asD = nc.dram_tensor("as_stream", [128, NCH * HEADS], bf16,
                         kind="ExternalInput")
    adD = nc.dram_tensor("ad_stream", [128, NCH * HEADS], bf16,
                         kind="ExternalInput")
    ohD = nc.dram_tensor("onehot", [128, NCH * TQ], fp8,
                         kind="ExternalInput")
    b1B = nc.dram_tensor("b1B", [128, HC], fp32, kind="ExternalInput")
    W2d = nc.dram_tensor("W2", [HC, HC], fp32, kind="ExternalInput")
    As2 = nc.dram_tensor("As2", [HC, HEADS], fp32, kind="ExternalInput")
    Ad2 = nc.dram_tensor("Ad2", [HC, HEADS], fp32, kind="ExternalInput")
    identD = nc.dram_tensor("ident128", [128, 128], fp32,
                            kind="ExternalInput")
    outD = nc.dram_tensor("hx2_loc", [SLAB, TW], bf16,
                          kind="ExternalOutput")

    with tile.TileContext(nc) as tc:
        with tc.tile_pool(name="const", bufs=1) as cp, \
             tc.tile_pool(name="sbH", bufs=3) as sbH, \
             tc.tile_pool(name="sbO", bufs=3) as sbO, \
             tc.tile_pool(name="sbP", bufs=2) as sbP, \
             tc.tile_pool(name="sbS", bufs=3) as sbS, \
             tc.tile_pool(name="sbA", bufs=3) as sbA, \
             tc.tile_pool(name="psA", bufs=2, space="PSUM") as psA, \
             tc.tile_pool(name="psU", bufs=2, space="PSUM") as psU:
            ident_t = cp.tile([128, 128], fp32)
            bias1 = cp.tile([128, HC], fp32)
            nc.sync.dma_start(out=ident_t[:], in_=identD[:])
            nc.sync.dma_start(out=bias1[:], in_=b1B[:])
            wfull2 = _build_wfull(nc, cp, psA, sbS, ident_t,
                                  W2d, As2, Ad2, mybir)

            def post(g, h1f):
                psT = psA.tile([128, 128], fp32, tag="psT2")
                nc.tensor.transpose(out=psT[:], in_=h1f[:],
                                    identity=ident_t[:])
                hT = sbA.tile([128, 128], bf16, tag="hT")
                nc.vector.tensor_copy(out=hT[:], in_=psT[:])
                psH = psA.tile([128, TW], fp32, tag="psH")
                nc.tensor.matmul(out=psH[:], lhsT=hT[:], rhs=wfull2[:],
                                 start=True, stop=True)
                hxt = sbA.tile([128, TW], bf16, tag="hxt")
                nc.vector.tensor_copy(out=hxt[:], in_=psH[:])
                nc.sync.dma_start(out=outD[g * 128:(g + 1) * 128, :],
                                  in_=hxt[:])

            _edge_layer(nc, (cp, sbH, sbO, sbP, sbS, psU), K,
                        hD, asD, adD, ohD, bias1, mybir, post)
    nc.compile()
    return nc


def _build_C(K):
    """Launch C: layer-2 edges -> pooling -> AllReduce -> heads."""
    bacc, mybir, tile, bass = _bass_mods()
    fp32 = mybir.dt.float32
    bf16 = mybir.dt.bfloat16
    fp8 = mybir.dt.float8e4
    OP = mybir.AluOpType
    NCH = TPC * K
    nc = bacc.Bacc("TRN2", target_bir_lowering=False, debug=False,
                   num_devices=C)
    hD = nc.dram_tensor("h_stream", [128, NCH * CW], bf16,
                        kind="ExternalInput")
    asD = nc.dram_tensor("as_stream", [128, NCH * HEADS], bf16,
                         kind="ExternalInput")
    adD = nc.dram_tensor("ad_stream", [128, NCH * HEADS], bf16,
                         kind="ExternalInput")
    ohD = nc.dram_tensor("onehot", [128, NCH * TQ], fp8,
                         kind="ExternalInput")
    b2B = nc.dram_tensor("b2B", [128, HC], fp32, kind="ExternalInput")
    poD = nc.dram_tensor("pool_onehot", [128, NG * G], bf16,
                         kind="ExternalInput")
    cntD = nc.dram_tensor("cnt", [G, 1], fp32, kind="ExternalInput")
    WrB = nc.dram_tensor("WrB", [G, HC], fp32, kind="ExternalInput")
    WtB = nc.dram_tensor("WtB", [G, HC], fp32, kind="ExternalInput")
    brB = nc.dram_tensor("brB", [G, 1], fp32, kind="ExternalInput")
    btB = nc.dram_tensor("btB", [G, 1], fp32, kind="ExternalInput")
    outD = nc.dram_tensor("out", [G, 2], fp32, kind="ExternalOutput")

    with tile.TileContext(nc) as tc:
        with tc.tile_pool(name="const", bufs=1) as cp, \
             tc.tile_pool(name="sbH", bufs=3) as sbH, \
             tc.tile_pool(name="sbO", bufs=3) as sbO, \
             tc.tile_pool(name="sbP", bufs=2) as sbP, \
             tc.tile_pool(name="sbS", bufs=4) as sbS, \
             tc.tile_pool(name="psU", bufs=2, space="PSUM") as psU, \
             tc.tile_pool(name="psP", bufs=1, space="PSUM") as psP, \
             tc.tile_pool(name="dram", bufs=1, space="DRAM") as dram:
            bias2 = cp.tile([128, HC], fp32)
            po_t = cp.tile([128, NG * G], bf16)
            nc.sync.dma_start(out=bias2[:], in_=b2B[:])
            nc.sync.dma_start(out=po_t[:], in_=poD[:])

            pool_ps = psP.tile([G, HC], fp32, tag="poolps", space="PSUM")

            def post(g, h1f):
                h2b = sbS.tile([128, HC], bf16, tag="h2b")
                nc.vector.tensor_copy(out=h2b[:], in_=h1f[:])
                nc.tensor.matmul(out=pool_ps[:],
                                 lhsT=po_t[:, g * G:(g + 1) * G],
                                 rhs=h2b[:],
                                 start=(g == 0), stop=(g == NG - 1))

            _edge_layer(nc, (cp, sbH, sbO, sbP, sbS, psU), K,
                        hD, asD, adD, ohD, bias2, mybir, post)

            pool_sb = sbS.tile([G, HC], fp32, tag="poolsb")
            nc.vector.tensor_copy(out=pool_sb[:], in_=pool_ps[:])
            ar_in = dram.tile([G, HC], fp32)
            ar_out = dram.tile([G, HC], fp32)
            nc.sync.dma_start(out=ar_in[:], in_=pool_sb[:])
            nc.gpsimd.collective_compute(
                "AllReduce", mybir.AluOpType.add,
                replica_groups=[list(range(C))],
                ins=[ar_in.opt()], outs=[ar_out.opt()])
            AR = sbS.tile([G, HC], fp32, tag="AR")
            nc.sync.dma_start(out=AR[:], in_=ar_out[:])

            WrT = cp.tile([G, HC], fp32)
            WtT = cp.tile([G, HC], fp32)
            brT = cp.tile([G, 1], fp32)
            btT = cp.tile([G, 1], fp32)
            cntT = cp.tile([G, 1], fp32)
            nc.sync.dma_start(out=WrT[:], in_=WrB[:])
            nc.sync.dma_start(out=WtT[:], in_=WtB[:])
            nc.sync.dma_start(out=brT[:], in_=brB[:])
            nc.sync.dma_start(out=btT[:], in_=btB[:])
            nc.sync.dma_start(out=cntT[:], in_=cntD[:])

            recC = sbS.tile([G, 1], fp32, tag="recC")
            nc.vector.reciprocal(recC[:], cntT[:])
            pooled = sbS.tile([G, HC], fp32, tag="pooled")
            nc.vector.tensor_tensor(out=pooled[:], in0=AR[:],
                                    in1=recC[:].to_broadcast([G, HC]),
                                    op=OP.mult)
            out_t = sbS.tile([G, 2], fp32, tag="outt")
            for j, Wt_ in enumerate([WrT, WtT]):
                prod = sbS.tile([G, HC], fp32, tag="prod")
                nc.vector.tensor_tensor(out=prod[:], in0=pooled[:],
                                        in1=Wt_[:], op=OP.mult)
                nc.vector.tensor_reduce(out=out_t[:, j:j + 1], in_=prod[:],
                                        axis=mybir.AxisListType.X, op=OP.add)
            nc.vector.tensor_tensor(out=out_t[:, 0:1], in0=out_t[:, 0:1],
                                    in1=brT[:], op=OP.add)
            nc.vector.tensor_tensor(out=out_t[:, 1:2], in0=out_t[:, 1:2],
                                    in1=btT[:], op=OP.add)
            nc.sync.dma_start(out=outD[:], in_=out_t[:])
    nc.compile()
    return nc


def _run(nc, in_maps, trace):
    from concourse.bass_utils import run_bass_kernel_spmd
    return run_bass_kernel_spmd(nc, in_maps, core_ids=list(range(C)),
                                trace=trace)


def kernel(**inputs):
    import ml_dtypes
    bf = ml_dtypes.bfloat16

    x = np.asarray(inputs["x"], np.float32)
    edge_index = np.asarray(inputs["edge_index"])
    batch = np.asarray(inputs["batch"])

    K, node_at, est, edt, oh_pc, po_pc, cnt = _preprocess(edge_index, batch)
    if _cache.get("K") != K:
        _cache.clear()
        _cache["K"] = K
        _cache["A"] = _build_A()
        _cache["B"] = _build_B(K)
        _cache["C"] = _build_C(K)
    ncA, ncB, ncC = _cache["A"], _cache["B"], _cache["C"]

    x_perm = np.zeros((NP, HC), np.float32)
    real = node_at >= 0
    x_perm[real] = x[node_at[real]]

    ident128 = np.eye(128, dtype=np.float32)
    b1B = np.ascontiguousarray(np.broadcast_to(
        np.asarray(inputs["b1"], np.float32), (128, HC)))
    b2B = np.ascontiguousarray(np.broadcast_to(
        np.asarray(inputs["b2"], np.float32), (128, HC)))
    WrB = np.ascontiguousarray(np.broadcast_to(
        np.asarray(inputs["Wr"], np.float32).reshape(1, HC), (G, HC)))
    WtB = np.ascontiguousarray(np.broadcast_to(
        np.asarray(inputs["Wt"], np.float32).reshape(1, HC), (G, HC)))
    brB = np.ascontiguousarray(np.broadcast_to(
        np.asarray(inputs["br"], np.float32).reshape(1, 1), (G, 1)))
    btB = np.ascontiguousarray(np.broadcast_to(
        np.asarray(inputs["bt"], np.float32).reshape(1, 1), (G, 1)))

    trace = os.environ.get("GAT_TRACE", "0") == "1"
    if trace:
        _install_ntff_shim()
    times = []

    # ---- launch A ----
    mapsA = []
    for c in range(C):
        xT = np.ascontiguousarray(
            x_perm[c * SLAB:(c + 1) * SLAB].T.astype(bf))
        mapsA.append({
            "xT": xT,
            "W1": np.asarray(inputs["W1"], np.float32),
            "As1": _block_att(inputs["att_src1"]),
            "Ad1": _block_att(inputs["att_dst1"]),
            "ident128": ident128,
        })
    resA = _run(ncA, mapsA, trace)
    times.append(resA.exec_time_ns)
    hx1 = np.concatenate([resA.results[c]["hx1_loc"] for c in range(C)])

    # ---- launch B ----
    mapsB = []
    for c in range(C):
        h_s, as_s, ad_s = _streams_for_core(hx1, est[c], edt[c])
        mapsB.append({
            "h_stream": h_s, "as_stream": as_s, "ad_stream": ad_s,
            "onehot": oh_pc[c], "b1B": b1B,
            "W2": np.asarray(inputs["W2"], np.float32),
            "As2": _block_att(inputs["att_src2"]),
            "Ad2": _block_att(inputs["att_dst2"]),
            "ident128": ident128,
        })
    resB = _run(ncB, mapsB, trace)
    times.append(resB.exec_time_ns)
    hx2 = np.concatenate([resB.results[c]["hx2_loc"] for c in range(C)])

    # ---- launch C ----
    mapsC = []
    for c in range(C):
        h_s, as_s, ad_s = _streams_for_core(hx2, est[c], edt[c])
        mapsC.append({
            "h_stream": h_s, "as_stream": as_s, "ad_stream": ad_s,
            "onehot": oh_pc[c], "b2B": b2B,
            "pool_onehot": po_pc[c], "cnt": cnt,
            "WrB": WrB, "WtB": WtB, "brB": brB, "btB": btB,
        })
    resC = _run(ncC, mapsC, trace)
    times.append(resC.exec_time_ns)

    kernel._last_exec_times_ns = times
    kernel._last_exec_time_ns = (sum(t for t in times if t is not None)
                                 if any(t is not None for t in times) else None)
    return np.asarray(resC.results[0]["out"])


kernel._last_exec_time_ns = None
kernel._last_exec_times_ns = None


def _install_ntff_shim():
    import types
    if "antenv.axon_hooks" in sys.modules:
        return
    try:
        from trn_agent_boot.trn_boot import _ntff_profile_via_ctypes
        hook = _ntff_profile_via_ctypes("/opt/axon/libaxon_pjrt.so")
    except Exception:
        hook = None
    mod = types.ModuleType("antenv.axon_hooks")
    mod.get_axon_ntff_profile_hook = lambda: hook
    mod.set_axon_ntff_profile_hook = lambda h: None
    sys.modules["antenv.axon_hooks"] = mod
